# revision 5
# baseline (speedup 1.0000x reference)
"""Trainium2 Bass kernel for GQA attention with RoPE and block-diagonal
(document) causal masking, sharded over 8 NeuronCores by KV head group.

Per core c (of 8): Q heads 4c..4c+3, KV head c, both batches.
Computation per core, all matmuls in float32r (tf32-grade, full PE rate):
  Q^T/K^T/V^T projections from host-transposed x^T, RoPE on Q/K (DVE),
  V^T transposed to V via PE, scores S^T = K^T_chunk.T-style layout
  [keys, queries], exp on ACT (scale fused), doc/causal masking via 0/1
  mask multiply (host-precomputed, only for partial tiles), PV matmul with
  an appended ones-column to get softmax denominators for free, normalize,
  o_proj partial, host sums the 8 partials.
"""
import sys
sys.path.insert(0, "/opt/trn_rl_repo")
import numpy as np

B, S, DIM = 2, 2048, 2048
NH, NKV, HD = 32, 8, 64
HPC = NH // 8           # 4 q-heads per core
MLOC = HPC * HD         # 256 local q dims
TQ = 256                # attention query chunk
NCORES = 8
NKC = DIM // 128        # 16 contraction chunks
NTC = S // 128          # 16 token chunks of 128
SCALE = 1.0 / 8.0

_nc_cache = {}


def _schedule(doc_ids):
    """Per batch: for each query chunk, the key-tile band and mask info.

    Returns sched[b][qc] = list of (kt, mask_index|None), masks array
    (P,128,TQ) float32 with 1.0 = attend, 0.0 = masked.
    """
    doc = np.asarray(doc_ids)
    sched = []
    masks = []
    for b in range(B):
        d = doc[b]
        # start index of each token's document
        change = np.empty(S, dtype=np.int64)
        change[0] = 0
        idx = np.arange(1, S)
        change[1:] = np.where(d[1:] != d[:-1], idx, 0)
        start_idx = np.maximum.accumulate(change)
        per_qc = []
        for qc in range(S // TQ):
            q0 = qc * TQ
            t0 = int(start_idx[q0]) // 128
            t1 = (q0 + TQ) // 128
            row = []
            for kt in range(t0, t1):
                k0 = kt * 128
                full = (k0 + 127 <= q0 and d[k0] == d[k0 + 127] == d[q0] == d[q0 + TQ - 1])
                if full:
                    row.append((kt, None))
                else:
                    ks = np.arange(k0, k0 + 128)
                    qs = np.arange(q0, q0 + TQ)
                    m = (d[ks][:, None] == d[qs][None, :]) & (ks[:, None] <= qs[None, :])
                    masks.append(m.astype(np.float32))
                    row.append((kt, len(masks) - 1))
            per_qc.append(row)
        sched.append(per_qc)
    if not masks:
        masks.append(np.zeros((128, TQ), np.float32))
    return sched, np.stack(masks)


def _build_nc(sched, nmask):
    import concourse.bacc as bacc
    import concourse.mybir as mybir
    import concourse.tile as tile
    from concourse.masks import make_identity

    F32, F32R = mybir.dt.float32, mybir.dt.float32r
    Exp = mybir.ActivationFunctionType.Exp

    nc = bacc.Bacc()
    xT = nc.dram_tensor("xT", (B, DIM, S), F32, kind="ExternalInput")
    wq = nc.dram_tensor("wq", (DIM, MLOC), F32, kind="ExternalInput")
    wkv = nc.dram_tensor("wkv", (DIM, 128), F32, kind="ExternalInput")
    wo = nc.dram_tensor("wo", (MLOC, DIM), F32, kind="ExternalInput")
    cos128 = nc.dram_tensor("cos128", (128, S), F32, kind="ExternalInput")
    sin128 = nc.dram_tensor("sin128", (128, S), F32, kind="ExternalInput")
    masks = nc.dram_tensor("masks", (nmask, 128, TQ), F32, kind="ExternalInput")
    y = nc.dram_tensor("y", (B, S, DIM), F32, kind="ExternalOutput")

    with tile.TileContext(nc) as tc:
        with (
            tc.tile_pool(name="const", bufs=1) as cst,
            tc.tile_pool(name="xt", bufs=3) as xtp,
            tc.tile_pool(name="big", bufs=1) as big,
            tc.tile_pool(name="rope", bufs=2) as rp,
            tc.tile_pool(name="pt", bufs=4) as ptp,
            tc.tile_pool(name="mask", bufs=2) as mp,
            tc.tile_pool(name="small", bufs=2) as sp,
            tc.tile_pool(name="ps", bufs=8, space="PSUM") as ps,
        ):
            # ---- constants ----
            wq_sb = cst.tile([128, NKC, MLOC], F32R)
            nc.sync.dma_start(wq_sb[:], wq[:].rearrange("(c p) m -> p c m", p=128).bitcast(F32R))
            wkv_sb = cst.tile([128, NKC, 128], F32R)
            nc.sync.dma_start(wkv_sb[:], wkv[:].rearrange("(c p) m -> p c m", p=128).bitcast(F32R))
            wo_sb = cst.tile([128, 2, DIM], F32R)
            nc.sync.dma_start(wo_sb[:], wo[:].rearrange("(c p) m -> p c m", p=128).bitcast(F32R))
            cos_sb = cst.tile([128, S], F32)
            nc.sync.dma_start(cos_sb[:], cos128[:])
            sin_sb = cst.tile([128, S], F32)
            nc.sync.dma_start(sin_sb[:], sin128[:])
            ident = cst.tile([128, 128], F32)
            make_identity(nc, ident[:])
            scratch1 = cst.tile([128, 1], F32)
            nc.gpsimd.memset(scratch1[:], 1.0)
            ones128 = cst.tile([128, 1], F32R)
            nc.vector.tensor_copy(ones128[:], scratch1[:])
            onesrow = cst.tile([1, 64], F32R)
            nc.vector.tensor_copy(onesrow[:], scratch1[0:1, 0:1].broadcast_to([1, 64]))

            for b in range(B):
                qrt = [big.tile([64, S], F32R, tag=f"qrt{m}", name=f"qrt{m}") for m in range(4)]
                krt = big.tile([64, S], F32R, tag="krt")
                vaug = big.tile([128, NTC, 65], F32R, tag="vaug")
                or2t = [big.tile([128, S], F32R, tag=f"or2t{m}", name=f"or2t{m}") for m in range(2)]

                # ================= projections + rope =================
                for th in range(2):
                    t0h = th * 1024
                    ps_q = [ps.tile([128, 512], F32, tag="ps", name=f"psq{b}{th}{i}") for i in range(4)]
                    ps_kv = [ps.tile([128, 512], F32, tag="ps", name=f"pskv{b}{th}{i}") for i in range(2)]
                    for kc in range(NKC):
                        xt = xtp.tile([128, 1024], F32R, tag="xt")
                        nc.sync.dma_start(
                            xt[:], xT[b, kc * 128:(kc + 1) * 128, t0h:t0h + 1024].bitcast(F32R))
                        st, sp_ = (kc == 0), (kc == NKC - 1)
                        for m in range(2):
                            for tt in range(2):
                                nc.tensor.matmul(
                                    ps_q[m * 2 + tt][:],
                                    wq_sb[:, kc, m * 128:(m + 1) * 128],
                                    xt[:, tt * 512:(tt + 1) * 512],
                                    start=st, stop=sp_)
                        for tt in range(2):
                            nc.tensor.matmul(
                                ps_kv[tt][:], wkv_sb[:, kc, :],
                                xt[:, tt * 512:(tt + 1) * 512], start=st, stop=sp_)

                    # rope on Q psums -> qrt
                    for m in range(2):
                        for tt in range(2):
                            pq = ps_q[m * 2 + tt]
                            t0 = t0h + tt * 512
                            tsl = slice(t0, t0 + 512)
                            tmp = rp.tile([128, 512], F32, tag="ra")
                            for blk in (0, 64):
                                nc.vector.tensor_mul(
                                    tmp[blk:blk + 32], pq[blk + 32:blk + 64],
                                    sin_sb[blk:blk + 32, tsl])
                                nc.vector.tensor_mul(
                                    tmp[blk + 32:blk + 64], pq[blk:blk + 32],
                                    sin_sb[blk + 32:blk + 64, tsl])
                            tmp2 = rp.tile([128, 512], F32, tag="rb")
                            nc.vector.tensor_mul(tmp2[:], pq[:], cos_sb[:, tsl])
                            nc.vector.tensor_add(qrt[2 * m][:, tsl], tmp[0:64], tmp2[0:64])
                            nc.vector.tensor_add(qrt[2 * m + 1][:, tsl], tmp[64:128], tmp2[64:128])

                    # rope on K (rows 0:64 of kv psum) -> krt ; V extract
                    for tt in range(2):
                        pkv = ps_kv[tt]
                        t0 = t0h + tt * 512
                        tsl = slice(t0, t0 + 512)
                        tmp = rp.tile([64, 512], F32, tag="rk")
                        nc.vector.tensor_mul(tmp[0:32], pkv[32:64], sin_sb[0:32, tsl])
                        nc.vector.tensor_mul(tmp[32:64], pkv[0:32], sin_sb[32:64, tsl])
                        tmp2 = rp.tile([64, 512], F32, tag="rl")
                        nc.vector.tensor_mul(tmp2[:], pkv[0:64], cos_sb[0:64, tsl])
                        nc.vector.tensor_add(krt[:, tsl], tmp[:], tmp2[:])
                        # V^T -> sbuf
                        vt = sp.tile([64, 512], F32, tag="vt")
                        nc.scalar.copy(vt[:], pkv[64:128])
                        for tc4 in range(4):
                            kt = (t0 // 128) + tc4
                            ptr = ps.tile([128, 64], F32, tag="ps")
                            nc.tensor.transpose(
                                ptr[:], vt[:, tc4 * 128:(tc4 + 1) * 128], ident[0:64, 0:64])
                            nc.vector.tensor_copy(vaug[:, kt, 0:64], ptr[:])
                            nc.vector.tensor_copy(vaug[:, kt, 64:65], ones128[:])

                # ================= attention =================
                for qc in range(S // TQ):
                    q0 = qc * TQ
                    row = sched[b][qc]
                    # prefetch masks for this qc (shared across the 4 heads)
                    mtiles = {}
                    for i, (kt, mi) in enumerate(row):
                        if mi is not None:
                            mt = mp.tile([128, TQ], F32R, tag=f"m{i}")
                            nc.sync.dma_start(mt[:], masks[mi].bitcast(F32R))
                            mtiles[kt] = mt
                    for h in range(HPC):
                        qtile = qrt[h]
                        hof = (h % 2) * 64
                        o_ps = ps.tile([65, TQ], F32, tag="ps")
                        for i, (kt, mi) in enumerate(row):
                            s_ps = ps.tile([128, TQ], F32, tag="ps")
                            nc.tensor.matmul(
                                s_ps[:], krt[:, kt * 128:(kt + 1) * 128],
                                qtile[:, q0:q0 + TQ],
                                start=True, stop=True)
                            pt = ptp.tile([128, TQ], F32R, tag="pt")
                            nc.scalar.activation(pt[:], s_ps[:], Exp, scale=SCALE)
                            if mi is not None:
                                nc.vector.tensor_mul(pt[:], pt[:], mtiles[kt][:])
                            nc.tensor.matmul(
                                o_ps[:], vaug[:, kt, :], pt[:],
                                start=(i == 0), stop=(i == len(row) - 1))
                        # normalize: rows 0:64 are O^T, row 64 is the denominator
                        rr = sp.tile([1, TQ], F32R, tag="rr")
                        with nc.allow_low_precision(reason="f32r view for PE broadcast"):
                            nc.vector.reciprocal(rr[:], o_ps[64:65, :])
                        zb_ps = ps.tile([64, TQ], F32, tag="ps")
                        nc.tensor.matmul(zb_ps[:], onesrow[:], rr[:], start=True, stop=True)
                        zb = sp.tile([64, TQ], F32, tag="zb")
                        nc.scalar.copy(zb[:], zb_ps[:])
                        nc.vector.tensor_mul(
                            or2t[h // 2][hof:hof + 64, q0:q0 + TQ], o_ps[0:64, :], zb[:])

                # ================= o_proj =================
                for tc_ in range(NTC):
                    for mc in range(4):
                        y_ps = ps.tile([128, 512], F32, tag="ps")
                        for hp in range(2):
                            nc.tensor.matmul(
                                y_ps[:], or2t[hp][:, tc_ * 128:(tc_ + 1) * 128],
                                wo_sb[:, hp, mc * 512:(mc + 1) * 512],
                                start=(hp == 0), stop=(hp == 1))
                        y_sb = sp.tile([128, 512], F32, tag="ysb")
                        nc.scalar.copy(y_sb[:], y_ps[:])
                        nc.sync.dma_start(
                            y[b, tc_ * 128:(tc_ + 1) * 128, mc * 512:(mc + 1) * 512],
                            y_sb[:])
    nc.finalize()
    return nc


def _prep_inputs(x, rope_cos, rope_sin, doc_ids, Wq, Wk, Wv, Wo):
    x = np.asarray(x, np.float32)
    xT = np.ascontiguousarray(x.transpose(0, 2, 1))
    cosT = np.asarray(rope_cos, np.float32).T          # (32, S)
    sinT = np.asarray(rope_sin, np.float32).T
    cos128 = np.tile(np.concatenate([cosT, cosT], 0), (2, 1))      # (128, S)
    sin128 = np.tile(np.concatenate([-sinT, sinT], 0), (2, 1))
    sched, masks = _schedule(doc_ids)
    Wq = np.asarray(Wq, np.float32)
    Wk = np.asarray(Wk, np.float32)
    Wv = np.asarray(Wv, np.float32)
    Wo = np.asarray(Wo, np.float32)
    in_maps = []
    for c in range(NCORES):
        wq_c = np.ascontiguousarray(Wq[c * MLOC:(c + 1) * MLOC].T)      # (DIM, 256)
        wk_c = Wk[c * HD:(c + 1) * HD].T                                # (DIM, 64)
        wv_c = Wv[c * HD:(c + 1) * HD].T
        wkv_c = np.ascontiguousarray(np.concatenate([wk_c, wv_c], 1))   # (DIM, 128)
        wo_c = np.ascontiguousarray(Wo[:, c * MLOC:(c + 1) * MLOC].T)   # (256, DIM)
        in_maps.append({
            "xT": xT, "wq": wq_c, "wkv": wkv_c, "wo": wo_c,
            "cos128": cos128, "sin128": sin128, "masks": masks,
        })
    return sched, masks, in_maps


def kernel(x, rope_cos, rope_sin, doc_ids, Wq, Wk, Wv, Wo):
    from concourse.bass_utils import run_bass_kernel_spmd
    sched, masks, in_maps = _prep_inputs(
        x, rope_cos, rope_sin, doc_ids, Wq, Wk, Wv, Wo)
    key = (tuple(tuple(tuple(r) for r in per_qc) for per_qc in
           [[tuple((kt, mi is not None) for kt, mi in row) for row in sb] for sb in sched]),
           masks.shape[0])
    nc = _nc_cache.get(key)
    if nc is None:
        nc = _build_nc(sched, masks.shape[0])
        _nc_cache[key] = nc
    res = run_bass_kernel_spmd(nc, in_maps, core_ids=list(range(NCORES)))
    y = np.zeros((B, S, DIM), np.float32)
    for c in range(NCORES):
        y += res.results[c]["y"]
    return y
